# revision 8
# baseline (speedup 1.0000x reference)
"""Trainium2 Bass kernel for the dual-modality dense transformer block.

Problem (hardcoded shapes): B=8, L=1024, H=512, NH=8, HD=64.
  - 6 linear projections (q/k/v for img and txt streams)
  - 4 full attentions: (q_img,KV_img), (q_txt,KV_txt), (q_img,KV_txt), (q_txt,KV_img)
  - out_img/out_txt linears on the averaged contexts, concat + cat linear
  - attention pooling (nn.MultiheadAttention-style) + out_proj

Sharding: pure data-parallel over batch B=8 across the 8 NeuronCores.

v2 changes over the original baseline (538us):
  - w_out_img/w_out_txt are folded into the two halves of w_cat on the host
    (W1 = Wc1@w_oim etc), so the averaged contexts feed the 'out' projection
    directly; the V-projection biases are likewise folded into the out bias
    (softmax-normalized probs sum to 1, so a V bias is a constant post-add).
  - softmax denominators: the ones-column of the augmented V is set to 2.0
    (folds the 0.5 averaging), the denominator row is DMA-reshaped to
    [128, 8] so the reciprocal is a 70ns DVE op instead of ~2.4us of
    single-partition work, then DMA-broadcast via DRAM as before.
  - a tunable subset of per-jt exp blocks runs on the Vector engine via a
    Schraudolph bit-trick (int32(A*x+B) then a bitcast cast to bf16), off
    the Scalar engine which is otherwise the bottleneck.
  - projection m-blocks are interleaved into attention phases as fillers.
"""

import numpy as np
import ml_dtypes

import concourse.bass as bass
import concourse.tile as tile
from concourse import bacc, mybir
from concourse.bass_utils import run_bass_kernel_spmd
from concourse.dve_ops import RECIP_APPROX_FAST_CONSTS, RECIPROCAL_APPROX_FAST

B, L, H, NH, HD = 8, 1024, 512, 8, 64
BF = mybir.dt.bfloat16
F32 = mybir.dt.float32
I32 = mybir.dt.int32
Exp = mybir.ActivationFunctionType.Exp
bf16 = ml_dtypes.bfloat16

N_CORES = 8

# Schraudolph exp constants: exp(x) ~= bitcast_f32(int32(A*x + Bc)).
# Bc centers the sawtooth error (+-2.98%, ~0 mean) so normalization
# cancels the common mode.
SCH_A = 8388608.0 / np.log(2.0)
SCH_B = 127.0 * 8388608.0 - 366392.0


def _emit(tc, d):
    nc = tc.nc
    import contextlib

    ctx = contextlib.ExitStack()
    with ctx:
        const = ctx.enter_context(tc.tile_pool(name="const", bufs=1))
        acts = ctx.enter_context(tc.tile_pool(name="acts", bufs=1))
        spool = ctx.enter_context(tc.tile_pool(name="spool", bufs=2))
        opool = ctx.enter_context(tc.tile_pool(name="opool", bufs=1))
        expool = ctx.enter_context(tc.tile_pool(name="expool", bufs=2))
        exipool = ctx.enter_context(tc.tile_pool(name="exi", bufs=2))
        small = ctx.enter_context(tc.tile_pool(name="small", bufs=2))
        dpool = ctx.enter_context(tc.tile_pool(name="dpool", bufs=3))
        dscr = ctx.enter_context(tc.tile_pool(name="dscr", bufs=4, space="DRAM"))
        pmm = ctx.enter_context(tc.tile_pool(name="pmm", bufs=2, space="PSUM"))
        pctx = ctx.enter_context(tc.tile_pool(name="pctx", bufs=2, space="PSUM"))

        # ---- constants / inputs into SBUF ----
        def load(name, p_chunks, free, dt=BF):
            t = const.tile([128, p_chunks, free], dt, tag=name)
            src_r = d[name].rearrange("(c p) n -> p c n", p=128)
            for c in range(p_chunks):
                nc.sync.dma_start(out=t[:, c, :], in_=src_r[:, c, :])
            return t

        def load_act(name, p_chunks, free, tag):
            t = acts.tile([128, p_chunks, free], BF, tag=tag)
            src_r = d[name].rearrange("(c p) n -> p c n", p=128)
            for c in range(p_chunks):
                nc.sync.dma_start(out=t[:, c, :], in_=src_r[:, c, :])
            return t

        def load2d(name, p, free, dt):
            t = const.tile([p, free], dt, tag=name)
            nc.sync.dma_start(out=t, in_=d[name])
            return t

        # load order matters: the first attention needs xT + img q/k/v only
        xt = load_act("xT", 4, L, "xT")
        w_qim = load("w_qim", 4, H)
        b_qim = load2d("b_qim", 128, 4, F32)
        w_kim = load("w_kim", 4, H)
        b_kim = load2d("b_kim", 128, 4, F32)
        w_vim = load("w_vim", 4, H)
        tt = load_act("tT", 4, L, "tT")
        w_qtx = load("w_qtx", 4, H)
        b_qtx = load2d("b_qtx", 128, 4, F32)
        w_ktx = load("w_ktx", 4, H)
        b_ktx = load2d("b_ktx", 128, 4, F32)
        w_vtx = load("w_vtx", 4, H)
        w_o1 = load("w_o1", 4, H)
        w_o2 = load("w_o2", 4, H)
        b_out = load2d("b_out", 128, 4, F32)
        w_ip = load("w_ip", 4, 3 * H)
        b_ipqk = load2d("b_ipqk", 128, 8, F32)
        w_op = load("w_op", 4, H)
        r_op = load2d("r_op", 1, H, BF)

        ones_row = const.tile([1, 128], BF, tag="ones_row")
        nc.vector.memset(ones_row, 1.0)

        # ---- helpers ----
        def proj_T_block(dst, m, src, nk, w, w_off, bias, bias_off):
            """One m-block (128 output features x all 1024 tokens) of a
            feature-major linear."""
            ps = pmm.tile([128, 1024], F32, tag="mm")
            for n in range(2):
                for k in range(nk):
                    nc.tensor.matmul(
                        ps[:, n * 512 : (n + 1) * 512],
                        w[:, k, w_off + m * 128 : w_off + (m + 1) * 128],
                        src[:, k, n * 512 : (n + 1) * 512],
                        start=(k == 0),
                        stop=(k == nk - 1),
                    )
            o = dst[:, m, :]
            if bias is not None:
                nc.vector.tensor_scalar_add(o, ps, bias[:, bias_off + m : bias_off + m + 1])
            else:
                nc.vector.tensor_copy(out=o, in_=ps)

        def proj_T(dst, src, nk, w, w_off, bias, bias_off):
            for m in range(4):
                proj_T_block(dst, m, src, nk, w, w_off, bias, bias_off)

        def proj_N_block(dst, lc2, src, w, w_off):
            """One 2-token-chunk block of the natural-orientation V linear
            into the ones-augmented layout [128, 8(lc), 8(head), 65]."""
            ps = pmm.tile([128, 1024], F32, tag="mm")
            for h in range(2):
                lc = lc2 * 2 + h
                for k in range(4):
                    nc.tensor.matmul(
                        ps[:, h * 512 : (h + 1) * 512],
                        src[:, k, lc * 128 : (lc + 1) * 128],
                        w[:, k, w_off : w_off + 512],
                        start=(k == 0),
                        stop=(k == 3),
                        skip_group_check=True,
                    )
            nc.vector.tensor_copy(
                out=dst[:, lc2 * 2 : lc2 * 2 + 2, :, 0:64],
                in_=ps.rearrange("p (a b) -> p a b", a=2),
            )

        # out projection: out_t[:, m, :] = sum_k W1T[m]@s_img[k] + W2T[m]@s_txt[k] + b
        def out_block(out_t, m, s_img, s_txt, half=None):
            nr = range(2) if half is None else [half]
            ps = pmm.tile([128, 1024], F32, tag="mm")
            for n in nr:
                for k in range(8):
                    srck = s_img if k < 4 else s_txt
                    wk = w_o1 if k < 4 else w_o2
                    nc.tensor.matmul(
                        ps[:, n * 512 : (n + 1) * 512],
                        wk[:, k % 4, m * 128 : (m + 1) * 128],
                        srck[:, k % 4, n * 512 : (n + 1) * 512],
                        start=(k == 0),
                        stop=(k == 7),
                    )
            for n in nr:
                nc.vector.tensor_scalar_add(
                    out_t[:, m, n * 512 : (n + 1) * 512],
                    ps[:, n * 512 : (n + 1) * 512],
                    b_out[:, m : m + 1],
                )

        # normalization closures, deferred one (p,ih) block (lag-1)
        pending = [None]

        def flush():
            if pending[0] is not None:
                pending[0]()
                pending[0] = None

        def attention(qT, kT, vN, s_dst, first, dve_jts=(), mid_hook=None):
            """One multi-head attention; accumulates normalized ctx' into s_dst.

            vN is ones-augmented [128, 8(jt), 8(head), 65]: col 64 of each head
            block holds the averaging scale (2.0, or 1.0 for the pooling
            attention) so the PV matmul emits scaled denominators on psum
            partition 64 for free.  dve_jts: jt indices whose exp runs on the
            Vector engine via the Schraudolph bit trick."""
            for ih in range(2):
                if ih == 1 and mid_hook is not None:
                    mid_hook()
                i0 = ih * 512
                for p in range(4):
                    ex = expool.tile([128, 8, 1024], BF, tag="exp")
                    cps = pctx.tile([128, 1024], F32, tag="ctx")
                    for jt in range(8):
                        ps = pmm.tile([128, 1024], F32, tag="mm")
                        for hh in range(2):
                            nc.tensor.matmul(
                                ps[:, hh * 512 : (hh + 1) * 512],
                                kT[hh * 64 : (hh + 1) * 64, p, jt * 128 : (jt + 1) * 128],
                                qT[hh * 64 : (hh + 1) * 64, p, i0 : i0 + 512],
                                start=True,
                                stop=True,
                                tile_position=(hh * 64, 0),
                            )
                        if jt in dve_jts:
                            exi = exipool.tile([128, 1024], I32, tag="exi")
                            nc.vector.tensor_scalar(
                                out=exi, in0=ps, scalar1=SCH_A, scalar2=SCH_B,
                                op0=mybir.AluOpType.mult, op1=mybir.AluOpType.add,
                            )
                            nc.vector.tensor_copy(
                                out=ex[:, jt, :], in_=exi.bitcast(F32)
                            )
                        else:
                            nc.scalar.activation(ex[:, jt, :], ps, Exp)
                        for hh in range(2):
                            nc.tensor.matmul(
                                cps[0:65, hh * 512 : (hh + 1) * 512],
                                vN[:, jt, p * 2 + hh, :],
                                ex[:, jt, hh * 512 : (hh + 1) * 512],
                                start=(jt == 0),
                                stop=(jt == 7),
                            )
                        if jt == 3:
                            flush()
                    # denominator chain: psum row 64 -> [128, 8] sbuf (DMA
                    # reshape), 70ns reciprocal, -> DRAM, -> partition-
                    # broadcast [64, 512] x2.  The ones-col scale already
                    # folded the 0.5 ctx averaging.
                    den_sb = dpool.tile([1, 1024], F32, tag="den_sb")
                    nc.vector.tensor_copy(out=den_sb, in_=cps[64:65, :])
                    den8 = dpool.tile([128, 8], F32, tag="den8")
                    nc.sync.dma_start(out=den8, in_=den_sb)
                    rc8 = dpool.tile([128, 8], BF, tag="rc8")
                    cdve = RECIP_APPROX_FAST_CONSTS
                    nc.vector._custom_dve(
                        RECIPROCAL_APPROX_FAST, out=rc8, in0=den8,
                        s0=cdve["s0"], s1=cdve["s1"], imm2=cdve["imm2"],
                    )
                    dr = dscr.tile([1, 1024], BF, tag="dr")
                    nc.sync.dma_start(
                        out=bass.AP(
                            tensor=dr.tensor, offset=dr.offset,
                            ap=[[8, 128], [1, 8]],
                        ),
                        in_=rc8,
                    )
                    bcs = dpool.tile([128, 512], BF, tag="bcs")
                    for hh in range(2):
                        sl = dr[0:1, hh * 512 : (hh + 1) * 512]
                        bsrc = bass.AP(tensor=sl.tensor, offset=sl.offset,
                                       ap=[[0, 64]] + [list(a) for a in sl.ap[1:]])
                        nc.sync.dma_start(out=bcs[hh * 64 : (hh + 1) * 64, :], in_=bsrc)

                    def normalize(cps=cps, bcs=bcs, p=p, i0=i0, first=first):
                        o = s_dst[:, p, i0 : i0 + 512]
                        if first:
                            nc.vector.tensor_mul(o[0:64, :], cps[0:64, 0:512], bcs[0:64, :])
                            nc.vector.tensor_mul(o[64:128, :], cps[0:64, 512:1024], bcs[64:128, :])
                        else:
                            tmp = small.tile([128, 512], BF, tag="tmp")
                            nc.vector.tensor_mul(tmp[0:64, :], cps[0:64, 0:512], bcs[0:64, :])
                            nc.vector.tensor_mul(tmp[64:128, :], cps[0:64, 512:1024], bcs[64:128, :])
                            nc.vector.tensor_add(o, o, tmp)

                    flush()
                    pending[0] = normalize

        # ---- the network ----
        q_im = acts.tile([128, 4, L], BF, tag="q_im")
        k_im = acts.tile([128, 4, L], BF, tag="k_im")
        v_im = acts.tile([128, 8, 8, 65], BF, tag="v_im")
        nc.vector.memset(v_im, 2.0)
        q_tx = acts.tile([128, 4, L], BF, tag="q_tx")
        k_tx = acts.tile([128, 4, L], BF, tag="k_tx")
        v_tx = acts.tile([128, 8, 8, 65], BF, tag="v_tx")
        nc.vector.memset(v_tx, 2.0)

        proj_T(q_im, xt, 4, w_qim, 0, b_qim, 0)
        proj_T(k_im, xt, 4, w_kim, 0, b_kim, 0)
        for lc2 in range(4):
            proj_N_block(v_im, lc2, xt, w_vim, 0)

        s_img = spool.tile([128, 4, L], BF, tag="s")
        s_txt = spool.tile([128, 4, L], BF, tag="s")

        attention(q_im, k_im, v_im, s_img, True)           # ctx_img

        for lc2 in range(4):
            proj_N_block(v_tx, lc2, tt, w_vtx, 0)
        proj_T(k_tx, tt, 4, w_ktx, 0, b_ktx, 0)
        proj_T(q_tx, tt, 4, w_qtx, 0, b_qtx, 0)

        attention(q_im, k_tx, v_tx, s_img, False, dve_jts=(7,))   # ctx_it
        attention(q_tx, k_tx, v_tx, s_txt, True, dve_jts=(7,))    # ctx_txt
        attention(q_tx, k_im, v_im, s_txt, False, dve_jts=(7,))   # ctx_ti
        flush()

        out_t = opool.tile([128, 4, L], BF, tag="out")
        for m in range(4):
            out_block(out_t, m, s_img, s_txt)

        q_pl = acts.tile([128, 4, L], BF, tag="q_im")
        k_pl = acts.tile([128, 4, L], BF, tag="q_tx")
        v_pl = acts.tile([128, 8, 8, 65], BF, tag="v_im")
        nc.vector.memset(v_pl, 1.0)
        proj_T(k_pl, out_t, 4, w_ip, 512, b_ipqk, 4)
        for lc2 in range(4):
            proj_N_block(v_pl, lc2, out_t, w_ip, 1024)
        proj_T(q_pl, out_t, 4, w_ip, 0, b_ipqk, 0)

        ctx_p = spool.tile([128, 4, L], BF, tag="s")

        def emit_out_proj(lcs):
            for lc in lcs:
                ps = pmm.tile([128, 1024], F32, tag="mm")
                for k in range(4):
                    nc.tensor.matmul(
                        ps[:, 0:512],
                        ctx_p[:, k, lc * 128 : (lc + 1) * 128],
                        w_op[:, k, :],
                        start=(k == 0),
                        stop=False,
                        skip_group_check=True,
                    )
                nc.tensor.matmul(
                    ps[:, 0:512], ones_row, r_op, start=False, stop=True,
                    skip_group_check=True,
                )
                res = small.tile([128, 512], F32, tag="res")
                nc.vector.tensor_copy(out=res, in_=ps[:, 0:512])
                nc.sync.dma_start(out=d["out"][lc * 128 : (lc + 1) * 128, :], in_=res)

        def pool_mid():
            flush()
            emit_out_proj(range(4))

        attention(q_pl, k_pl, v_pl, ctx_p, True, dve_jts=(7,), mid_hook=pool_mid)
        flush()
        emit_out_proj(range(4, 8))


_PROGRAM = None


def _build_program():
    global _PROGRAM
    if _PROGRAM is not None:
        return _PROGRAM
    nc = bacc.Bacc("TRN2", target_bir_lowering=False, debug=False)
    d = {}

    def din(name, shape, dt):
        d[name] = nc.dram_tensor(name, list(shape), dt, kind="ExternalInput").ap()

    din("xT", (H, L), BF)
    din("tT", (H, L), BF)
    for n in ("w_qim", "w_kim", "w_vim", "w_qtx", "w_ktx", "w_vtx", "w_o1", "w_o2"):
        din(n, (H, H), BF)
    din("w_ip", (H, 3 * H), BF)
    din("w_op", (H, H), BF)
    for n in ("b_qim", "b_kim", "b_qtx", "b_ktx", "b_out"):
        din(n, (128, 4), F32)
    din("b_ipqk", (128, 8), F32)
    din("r_op", (1, H), BF)
    d["out"] = nc.dram_tensor("out", [L, H], F32, kind="ExternalOutput").ap()

    with tile.TileContext(nc) as tc:
        _emit(tc, d)
    nc.compile()
    _PROGRAM = nc
    return nc


def _host_prep(inputs):
    f = lambda x: np.asarray(x, np.float32)

    def wT(w, scale=None):
        w = f(w)
        if scale is not None:
            w = w * scale
        return np.ascontiguousarray(w.T).astype(bf16)

    def bcol(b, scale=None):
        b = f(b)
        if scale is not None:
            b = b * scale
        return np.ascontiguousarray(b.reshape(-1, 128).T.astype(np.float32))

    s = 1.0 / np.sqrt(HD)
    ipw = f(inputs["in_proj_w"]).copy()
    ipw[0:H] *= s
    ipb = f(inputs["in_proj_b"]).copy()
    ipb[0:H] *= s

    # fold out_img/out_txt + their biases + the V biases into the cat linear
    wc = f(inputs["w_cat"])
    wc1, wc2 = wc[:, 0:H], wc[:, H : 2 * H]
    w1 = wc1 @ f(inputs["w_out_img"])
    w2 = wc2 @ f(inputs["w_out_txt"])
    cv = 0.5 * (f(inputs["b_v_img"]) + f(inputs["b_v_txt"]))
    b_out = (
        f(inputs["b_cat"])
        + wc1 @ f(inputs["b_out_img"])
        + wc2 @ f(inputs["b_out_txt"])
        + w1 @ cv
        + w2 @ cv
    )

    shared = {
        "w_qim": wT(inputs["w_q_img"], s),
        "w_kim": wT(inputs["w_k_img"]),
        "w_vim": wT(inputs["w_v_img"]),
        "w_qtx": wT(inputs["w_q_txt"], s),
        "w_ktx": wT(inputs["w_k_txt"]),
        "w_vtx": wT(inputs["w_v_txt"]),
        "w_o1": wT(w1),
        "w_o2": wT(w2),
        "w_ip": wT(ipw),
        "w_op": wT(inputs["out_proj_w"]),
        "b_qim": bcol(inputs["b_q_img"], s),
        "b_kim": bcol(inputs["b_k_img"]),
        "b_qtx": bcol(inputs["b_q_txt"], s),
        "b_ktx": bcol(inputs["b_k_txt"]),
        "b_out": bcol(b_out),
        "b_ipqk": bcol(ipb[0 : 2 * H]),
        "r_op": f(inputs["out_proj_b"]).astype(bf16).reshape(1, -1),
    }
    hs = f(inputs["hidden_states"])
    tx = f(inputs["text"])
    in_maps = []
    for c in range(N_CORES):
        m = dict(shared)
        m["xT"] = np.ascontiguousarray(hs[c].T).astype(bf16)
        m["tT"] = np.ascontiguousarray(tx[c].T).astype(bf16)
        in_maps.append(m)
    return in_maps


def kernel(**inputs):
    nc = _build_program()
    in_maps = _host_prep(inputs)
    res = run_bass_kernel_spmd(nc, in_maps, core_ids=list(range(N_CORES)))
    out = np.stack([res.results[c]["out"] for c in range(N_CORES)])
    return out.astype(np.float32)


# revision 10
# speedup vs baseline: 1.1899x; 1.1899x over previous
"""Trainium2 Bass kernel for the dual-modality dense transformer block.

Problem (hardcoded shapes): B=8, L=1024, H=512, NH=8, HD=64.
  - 6 linear projections (q/k/v for img and txt streams)
  - 4 full attentions: (q_img,KV_img), (q_txt,KV_txt), (q_img,KV_txt), (q_txt,KV_img)
  - out_img/out_txt linears on the averaged contexts, concat + cat linear
  - attention pooling (nn.MultiheadAttention-style) + out_proj

Sharding: pure data-parallel over batch B=8 across the 8 NeuronCores.

v2 changes over the original baseline (538us):
  - w_out_img/w_out_txt are folded into the two halves of w_cat on the host
    (W1 = Wc1@w_oim etc), so the averaged contexts feed the 'out' projection
    directly; the V-projection biases are likewise folded into the out bias
    (softmax-normalized probs sum to 1, so a V bias is a constant post-add).
  - softmax denominators: the ones-column of the augmented V is set to 2.0
    (folds the 0.5 averaging), the denominator row is DMA-reshaped to
    [128, 8] so the reciprocal is a 70ns DVE op instead of ~2.4us of
    single-partition work, then DMA-broadcast via DRAM as before.
  - a tunable subset of per-jt exp blocks runs on the Vector engine via a
    Schraudolph bit-trick (int32(A*x+B) then a bitcast cast to bf16), off
    the Scalar engine which is otherwise the bottleneck.
  - projection m-blocks are interleaved into attention phases as fillers.
"""

import numpy as np
import ml_dtypes

import concourse.bass as bass
import concourse.tile as tile
from concourse import bacc, mybir
from concourse.bass_utils import run_bass_kernel_spmd
from concourse.dve_ops import RECIP_APPROX_FAST_CONSTS, RECIPROCAL_APPROX_FAST

B, L, H, NH, HD = 8, 1024, 512, 8, 64
BF = mybir.dt.bfloat16
F32 = mybir.dt.float32
I32 = mybir.dt.int32
Exp = mybir.ActivationFunctionType.Exp
bf16 = ml_dtypes.bfloat16

N_CORES = 8

# Schraudolph exp constants: exp(x) ~= bitcast_f32(int32(A*x + Bc)).
# Bc centers the sawtooth error (+-2.98%, ~0 mean) so normalization
# cancels the common mode.
SCH_A = 8388608.0 / np.log(2.0)
SCH_B = 127.0 * 8388608.0 - 366392.0


def _emit(tc, d):
    nc = tc.nc
    import contextlib

    ctx = contextlib.ExitStack()
    with ctx:
        const = ctx.enter_context(tc.tile_pool(name="const", bufs=1))
        acts = ctx.enter_context(tc.tile_pool(name="acts", bufs=1))
        spool = ctx.enter_context(tc.tile_pool(name="spool", bufs=2))
        opool = ctx.enter_context(tc.tile_pool(name="opool", bufs=1))
        expool = ctx.enter_context(tc.tile_pool(name="expool", bufs=2))
        exipool = ctx.enter_context(tc.tile_pool(name="exi", bufs=2))
        small = ctx.enter_context(tc.tile_pool(name="small", bufs=2))
        dpool = ctx.enter_context(tc.tile_pool(name="dpool", bufs=3))
        dscr = ctx.enter_context(tc.tile_pool(name="dscr", bufs=4, space="DRAM"))
        pmm = ctx.enter_context(tc.tile_pool(name="pmm", bufs=2, space="PSUM"))
        pctx = ctx.enter_context(tc.tile_pool(name="pctx", bufs=2, space="PSUM"))

        # ---- constants / inputs into SBUF ----
        def load(name, p_chunks, free, dt=BF):
            t = const.tile([128, p_chunks, free], dt, tag=name)
            src_r = d[name].rearrange("(c p) n -> p c n", p=128)
            for c in range(p_chunks):
                nc.sync.dma_start(out=t[:, c, :], in_=src_r[:, c, :])
            return t

        def load_act(name, p_chunks, free, tag):
            t = acts.tile([128, p_chunks, free], BF, tag=tag)
            src_r = d[name].rearrange("(c p) n -> p c n", p=128)
            for c in range(p_chunks):
                nc.sync.dma_start(out=t[:, c, :], in_=src_r[:, c, :])
            return t

        def load2d(name, p, free, dt):
            t = const.tile([p, free], dt, tag=name)
            nc.sync.dma_start(out=t, in_=d[name])
            return t

        # load order matters: the first attention needs xT + img q/k/v only
        xt = load_act("xT", 4, L, "xT")
        w_qim = load("w_qim", 4, H)
        b_qim = load2d("b_qim", 128, 4, F32)
        w_kim = load("w_kim", 4, H)
        b_kim = load2d("b_kim", 128, 4, F32)
        w_vim = load("w_vim", 4, H)
        tt = load_act("tT", 4, L, "tT")
        w_qtx = load("w_qtx", 4, H)
        b_qtx = load2d("b_qtx", 128, 4, F32)
        w_ktx = load("w_ktx", 4, H)
        b_ktx = load2d("b_ktx", 128, 4, F32)
        w_vtx = load("w_vtx", 4, H)
        w_o1 = load("w_o1", 4, H)
        w_o2 = load("w_o2", 4, H)
        b_out = load2d("b_out", 128, 4, F32)
        w_ip = load("w_ip", 4, 3 * H)
        b_ipqk = load2d("b_ipqk", 128, 8, F32)
        w_op = load("w_op", 4, H)
        r_op = load2d("r_op", 1, H, BF)

        ones_row = const.tile([1, 128], BF, tag="ones_row")
        nc.vector.memset(ones_row, 1.0)

        # ---- helpers ----
        def proj_T_block(dst, m, src, nk, w, w_off, bias, bias_off):
            """One m-block (128 output features x all 1024 tokens) of a
            feature-major linear."""
            ps = pmm.tile([128, 1024], F32, tag="mm")
            for n in range(2):
                for k in range(nk):
                    nc.tensor.matmul(
                        ps[:, n * 512 : (n + 1) * 512],
                        w[:, k, w_off + m * 128 : w_off + (m + 1) * 128],
                        src[:, k, n * 512 : (n + 1) * 512],
                        start=(k == 0),
                        stop=(k == nk - 1),
                    )
            o = dst[:, m, :]
            if bias is not None:
                nc.vector.tensor_scalar_add(o, ps, bias[:, bias_off + m : bias_off + m + 1])
            else:
                nc.vector.tensor_copy(out=o, in_=ps)

        def proj_T(dst, src, nk, w, w_off, bias, bias_off):
            for m in range(4):
                proj_T_block(dst, m, src, nk, w, w_off, bias, bias_off)

        def proj_N_block(dst, lc2, src, w, w_off):
            """One 2-token-chunk block of the natural-orientation V linear
            into the ones-augmented layout [128, 8(lc), 8(head), 65]."""
            ps = pmm.tile([128, 1024], F32, tag="mm")
            for h in range(2):
                lc = lc2 * 2 + h
                for k in range(4):
                    nc.tensor.matmul(
                        ps[:, h * 512 : (h + 1) * 512],
                        src[:, k, lc * 128 : (lc + 1) * 128],
                        w[:, k, w_off : w_off + 512],
                        start=(k == 0),
                        stop=(k == 3),
                        skip_group_check=True,
                    )
            nc.vector.tensor_copy(
                out=dst[:, lc2 * 2 : lc2 * 2 + 2, :, 0:64],
                in_=ps.rearrange("p (a b) -> p a b", a=2),
            )

        # out projection: out_t[:, m, :] = sum_k W1T[m]@s_img[k] + W2T[m]@s_txt[k] + b
        def out_block(out_t, m, s_img, s_txt, half=None):
            nr = range(2) if half is None else [half]
            ps = pmm.tile([128, 1024], F32, tag="mm")
            for n in nr:
                for k in range(8):
                    srck = s_img if k < 4 else s_txt
                    wk = w_o1 if k < 4 else w_o2
                    nc.tensor.matmul(
                        ps[:, n * 512 : (n + 1) * 512],
                        wk[:, k % 4, m * 128 : (m + 1) * 128],
                        srck[:, k % 4, n * 512 : (n + 1) * 512],
                        start=(k == 0),
                        stop=(k == 7),
                    )
            for n in nr:
                nc.vector.tensor_scalar_add(
                    out_t[:, m, n * 512 : (n + 1) * 512],
                    ps[:, n * 512 : (n + 1) * 512],
                    b_out[:, m : m + 1],
                )

        # normalization closures, deferred one (p,ih) block (lag-1)
        pending = [None]

        def flush():
            if pending[0] is not None:
                pending[0]()
                pending[0] = None

        def attention(qT, kT, vN, s_dst, first, dve_jts=(), mid_hook=None):
            """One multi-head attention; accumulates normalized ctx' into s_dst.

            vN is ones-augmented [128, 8(jt), 8(head), 65]: col 64 of each head
            block holds the averaging scale (2.0, or 1.0 for the pooling
            attention) so the PV matmul emits scaled denominators on psum
            partition 64 for free.  dve_jts: jt indices whose exp runs on the
            Vector engine via the Schraudolph bit trick."""
            for ih in range(2):
                if ih == 1 and mid_hook is not None:
                    mid_hook()
                i0 = ih * 512
                for p in range(4):
                    ex = expool.tile([128, 8, 1024], BF, tag="exp")
                    # scores (transposed), both heads into one 2-bank tile
                    for jt in range(8):
                        ps = pmm.tile([128, 1024], F32, tag="mm")
                        for hh in range(2):
                            nc.tensor.matmul(
                                ps[:, hh * 512 : (hh + 1) * 512],
                                kT[hh * 64 : (hh + 1) * 64, p, jt * 128 : (jt + 1) * 128],
                                qT[hh * 64 : (hh + 1) * 64, p, i0 : i0 + 512],
                                start=True,
                                stop=True,
                                tile_position=(hh * 64, 0),
                            )
                        if jt in dve_jts:
                            exi = exipool.tile([128, 1024], I32, tag="exi")
                            nc.vector.tensor_scalar(
                                out=exi, in0=ps, scalar1=SCH_A, scalar2=SCH_B,
                                op0=mybir.AluOpType.mult, op1=mybir.AluOpType.add,
                            )
                            nc.vector.tensor_copy(
                                out=ex[:, jt, :], in_=exi.bitcast(F32)
                            )
                        else:
                            nc.scalar.activation(ex[:, jt, :], ps, Exp)
                    # PV + denominators
                    cps = pctx.tile([128, 1024], F32, tag="ctx")
                    for jt in range(8):
                        for hh in range(2):
                            nc.tensor.matmul(
                                cps[0:65, hh * 512 : (hh + 1) * 512],
                                vN[:, jt, p * 2 + hh, :],
                                ex[:, jt, hh * 512 : (hh + 1) * 512],
                                start=(jt == 0),
                                stop=(jt == 7),
                            )
                    flush()

                    def normalize(cps=cps, p=p, i0=i0, first=first):
                        den = small.tile([1, 1024], F32, tag="den")
                        nc.vector.tensor_copy(out=den, in_=cps[64:65, :])
                        rc = small.tile([1, 1024], BF, tag="rc")
                        cdve = RECIP_APPROX_FAST_CONSTS
                        nc.vector._custom_dve(
                            RECIPROCAL_APPROX_FAST, out=rc, in0=den,
                            s0=cdve["s0"], s1=cdve["s1"], imm2=cdve["imm2"],
                        )
                        dr = dscr.tile([1, 1024], BF, tag="dr")
                        nc.sync.dma_start(out=dr, in_=rc)
                        bcs = small.tile([128, 512], BF, tag="bcs")
                        for hh in range(2):
                            sl = dr[0:1, hh * 512 : (hh + 1) * 512]
                            bsrc = bass.AP(tensor=sl.tensor, offset=sl.offset,
                                           ap=[[0, 64]] + [list(a) for a in sl.ap[1:]])
                            nc.sync.dma_start(out=bcs[hh * 64 : (hh + 1) * 64, :], in_=bsrc)
                        o = s_dst[:, p, i0 : i0 + 512]
                        if first:
                            nc.vector.tensor_mul(o[0:64, :], cps[0:64, 0:512], bcs[0:64, :])
                            nc.vector.tensor_mul(o[64:128, :], cps[0:64, 512:1024], bcs[64:128, :])
                        else:
                            tmp = small.tile([128, 512], BF, tag="tmp")
                            nc.vector.tensor_mul(tmp[0:64, :], cps[0:64, 0:512], bcs[0:64, :])
                            nc.vector.tensor_mul(tmp[64:128, :], cps[0:64, 512:1024], bcs[64:128, :])
                            nc.vector.tensor_add(o, o, tmp)

                    pending[0] = normalize

        # ---- the network ----
        q_im = acts.tile([128, 4, L], BF, tag="q_im")
        k_im = acts.tile([128, 4, L], BF, tag="k_im")
        v_im = acts.tile([128, 8, 8, 65], BF, tag="v_im")
        nc.vector.memset(v_im, 2.0)
        q_tx = acts.tile([128, 4, L], BF, tag="q_tx")
        k_tx = acts.tile([128, 4, L], BF, tag="k_tx")
        v_tx = acts.tile([128, 8, 8, 65], BF, tag="v_tx")
        nc.vector.memset(v_tx, 2.0)

        proj_T(q_im, xt, 4, w_qim, 0, b_qim, 0)
        proj_T(k_im, xt, 4, w_kim, 0, b_kim, 0)
        for lc2 in range(4):
            proj_N_block(v_im, lc2, xt, w_vim, 0)

        s_img = spool.tile([128, 4, L], BF, tag="s")
        s_txt = spool.tile([128, 4, L], BF, tag="s")

        attention(q_im, k_im, v_im, s_img, True)           # ctx_img

        for lc2 in range(4):
            proj_N_block(v_tx, lc2, tt, w_vtx, 0)
        proj_T(k_tx, tt, 4, w_ktx, 0, b_ktx, 0)
        proj_T(q_tx, tt, 4, w_qtx, 0, b_qtx, 0)

        attention(q_im, k_tx, v_tx, s_img, False)   # ctx_it
        attention(q_tx, k_tx, v_tx, s_txt, True)    # ctx_txt
        attention(q_tx, k_im, v_im, s_txt, False)   # ctx_ti
        flush()

        out_t = opool.tile([128, 4, L], BF, tag="out")
        for m in range(4):
            out_block(out_t, m, s_img, s_txt)

        q_pl = acts.tile([128, 4, L], BF, tag="q_im")
        k_pl = acts.tile([128, 4, L], BF, tag="q_tx")
        v_pl = acts.tile([128, 8, 8, 65], BF, tag="v_im")
        nc.vector.memset(v_pl, 1.0)
        proj_T(k_pl, out_t, 4, w_ip, 512, b_ipqk, 4)
        for lc2 in range(4):
            proj_N_block(v_pl, lc2, out_t, w_ip, 1024)
        proj_T(q_pl, out_t, 4, w_ip, 0, b_ipqk, 0)

        ctx_p = spool.tile([128, 4, L], BF, tag="s")

        def emit_out_proj(lcs):
            for lc in lcs:
                ps = pmm.tile([128, 1024], F32, tag="mm")
                for k in range(4):
                    nc.tensor.matmul(
                        ps[:, 0:512],
                        ctx_p[:, k, lc * 128 : (lc + 1) * 128],
                        w_op[:, k, :],
                        start=(k == 0),
                        stop=False,
                        skip_group_check=True,
                    )
                nc.tensor.matmul(
                    ps[:, 0:512], ones_row, r_op, start=False, stop=True,
                    skip_group_check=True,
                )
                res = small.tile([128, 512], F32, tag="res")
                nc.vector.tensor_copy(out=res, in_=ps[:, 0:512])
                nc.sync.dma_start(out=d["out"][lc * 128 : (lc + 1) * 128, :], in_=res)

        def pool_mid():
            flush()
            emit_out_proj(range(4))

        attention(q_pl, k_pl, v_pl, ctx_p, True, mid_hook=pool_mid)
        flush()
        emit_out_proj(range(4, 8))


_PROGRAM = None


def _build_program():
    global _PROGRAM
    if _PROGRAM is not None:
        return _PROGRAM
    nc = bacc.Bacc("TRN2", target_bir_lowering=False, debug=False)
    d = {}

    def din(name, shape, dt):
        d[name] = nc.dram_tensor(name, list(shape), dt, kind="ExternalInput").ap()

    din("xT", (H, L), BF)
    din("tT", (H, L), BF)
    for n in ("w_qim", "w_kim", "w_vim", "w_qtx", "w_ktx", "w_vtx", "w_o1", "w_o2"):
        din(n, (H, H), BF)
    din("w_ip", (H, 3 * H), BF)
    din("w_op", (H, H), BF)
    for n in ("b_qim", "b_kim", "b_qtx", "b_ktx", "b_out"):
        din(n, (128, 4), F32)
    din("b_ipqk", (128, 8), F32)
    din("r_op", (1, H), BF)
    d["out"] = nc.dram_tensor("out", [L, H], F32, kind="ExternalOutput").ap()

    with tile.TileContext(nc) as tc:
        _emit(tc, d)
    nc.compile()
    _PROGRAM = nc
    return nc


def _host_prep(inputs):
    f = lambda x: np.asarray(x, np.float32)

    def wT(w, scale=None):
        w = f(w)
        if scale is not None:
            w = w * scale
        return np.ascontiguousarray(w.T).astype(bf16)

    def bcol(b, scale=None):
        b = f(b)
        if scale is not None:
            b = b * scale
        return np.ascontiguousarray(b.reshape(-1, 128).T.astype(np.float32))

    s = 1.0 / np.sqrt(HD)
    ipw = f(inputs["in_proj_w"]).copy()
    ipw[0:H] *= s
    ipb = f(inputs["in_proj_b"]).copy()
    ipb[0:H] *= s

    # fold out_img/out_txt + their biases + the V biases into the cat linear
    wc = f(inputs["w_cat"])
    wc1, wc2 = wc[:, 0:H], wc[:, H : 2 * H]
    w1 = wc1 @ f(inputs["w_out_img"])
    w2 = wc2 @ f(inputs["w_out_txt"])
    cv = 0.5 * (f(inputs["b_v_img"]) + f(inputs["b_v_txt"]))
    b_out = (
        f(inputs["b_cat"])
        + wc1 @ f(inputs["b_out_img"])
        + wc2 @ f(inputs["b_out_txt"])
        + w1 @ cv
        + w2 @ cv
    )

    shared = {
        "w_qim": wT(inputs["w_q_img"], s),
        "w_kim": wT(inputs["w_k_img"]),
        "w_vim": wT(inputs["w_v_img"]),
        "w_qtx": wT(inputs["w_q_txt"], s),
        "w_ktx": wT(inputs["w_k_txt"]),
        "w_vtx": wT(inputs["w_v_txt"]),
        "w_o1": wT(w1),
        "w_o2": wT(w2),
        "w_ip": wT(ipw),
        "w_op": wT(inputs["out_proj_w"]),
        "b_qim": bcol(inputs["b_q_img"], s),
        "b_kim": bcol(inputs["b_k_img"]),
        "b_qtx": bcol(inputs["b_q_txt"], s),
        "b_ktx": bcol(inputs["b_k_txt"]),
        "b_out": bcol(b_out),
        "b_ipqk": bcol(ipb[0 : 2 * H]),
        "r_op": f(inputs["out_proj_b"]).astype(bf16).reshape(1, -1),
    }
    hs = f(inputs["hidden_states"])
    tx = f(inputs["text"])
    in_maps = []
    for c in range(N_CORES):
        m = dict(shared)
        m["xT"] = np.ascontiguousarray(hs[c].T).astype(bf16)
        m["tT"] = np.ascontiguousarray(tx[c].T).astype(bf16)
        in_maps.append(m)
    return in_maps


def kernel(**inputs):
    nc = _build_program()
    in_maps = _host_prep(inputs)
    res = run_bass_kernel_spmd(nc, in_maps, core_ids=list(range(N_CORES)))
    out = np.stack([res.results[c]["out"] for c in range(N_CORES)])
    return out.astype(np.float32)
